# revision 49
# baseline (speedup 1.0000x reference)
"""Trainium2 Bass kernel for nn_CartTensorOut (e3nn-style CartTensorOut layer).

Strategy (v2):
- Data-parallel over nodes: 20000 nodes -> 8 cores x 2500; 5 chunks of 500.
- Host folds post_lin vectors P into Wm2, collapsing the tensor product +
  post_lin to an 18-block bilinear form (paths 3/7 vanish, 1/5 merge).
- Device: h-planes are produced by PE matmuls laid out so every pair
  product is a partition-offset view of just three evicted planes
  (H0R|H_B|H2) -- no shift copies or shuffling DMAs at all.
- PSUM is managed as 2 alternating 4-bank "quads" (h-quad, pw-quad,
  G-quad per chunk); wide 2-bank evictions amortize fixed op costs.
- v = wp * G products run on DVE as two wide bf16 ops; the reduction to
  z[6] and the sph->cart->segment_sum finish on PE / host as before.
"""
import numpy as np
import ml_dtypes

BF = ml_dtypes.bfloat16
N_NODES = 20000
N_GRAPH = 256
N_CORES = 8
NC_PER = N_NODES // N_CORES          # 2500
F = 500                               # nodes per chunk
CHUNKS = [500, 500, 500, 500, 500]
NCHUNK = len(CHUNKS)
COFF = [sum(CHUNKS[:i]) for i in range(NCHUNK)]
HC = 32
N_PATH = 9


# ---------------- constant tables (from reference's cartesian tensor algebra) ----
def _tables():
    eps = np.zeros((3, 3, 3))
    eps[0, 1, 2] = eps[1, 2, 0] = eps[2, 0, 1] = 1.0
    eps[0, 2, 1] = eps[2, 1, 0] = eps[1, 0, 2] = -1.0
    s2, s6 = 1 / np.sqrt(2), 1 / np.sqrt(6)
    Q2 = np.zeros((5, 3, 3))
    Q2[0, 0, 1] = Q2[0, 1, 0] = s2
    Q2[1, 1, 2] = Q2[1, 2, 1] = s2
    Q2[2, 0, 0] = Q2[2, 1, 1] = -s6; Q2[2, 2, 2] = 2 * s6
    Q2[3, 0, 2] = Q2[3, 2, 0] = s2
    Q2[4, 0, 0] = s2; Q2[4, 1, 1] = -s2

    def nrm(C, l3):
        return C * np.sqrt((2 * l3 + 1) / (C ** 2).sum())

    M = np.einsum('iab,jdb->ijad', Q2, Q2)
    S = 0.5 * (M + np.transpose(M, (0, 1, 3, 2)))
    S = S - np.trace(S, axis1=2, axis2=3)[..., None, None] * np.eye(3) / 3.0
    C222 = nrm(np.einsum('kad,ijad->ijk', Q2, S), 2)
    Qc = np.zeros((3, 3, 9))
    Qc[:, :, 0] = np.eye(3) / np.sqrt(3)
    Qc[:, :, 1:4] = eps / np.sqrt(2)
    Qc[:, :, 4:9] = np.transpose(Q2, (1, 2, 0))
    return C222, Qc


C222, QC = _tables()


# ---------------- host-side weight folding ----------------
def fold_weights(W0e, W1o, W2e, Wm1, bm1, Wm2, bm2, P0, P1, P2):
    f = {}
    W0 = W0e / np.sqrt(128)
    W1 = W1o / np.sqrt(64)
    W2 = W2e / np.sqrt(32)
    z = np.zeros
    f['Wh0R'] = np.tile(W0, (1, 4)).astype(np.float32)      # [128,128] -> [h0 x4]
    f['Wh101'] = np.block([[W1, z((64, 32))], [z((64, 32)), W1]]).astype(np.float32)
    # WhZS4: K=96 rows [x1m2(64), x2r4(32)] -> outputs [z, s4]
    f['WhZS4'] = np.block([
        [W1, z((64, 32))],
        [z((32, 32)), W2],
    ]).astype(np.float32)
    # WhS: K=128 rows [x2r0..x2r3] -> outputs [s0..s3]
    f['WhS'] = np.block([
        [W2, z((32, 32)), z((32, 32)), z((32, 32))],
        [z((32, 32)), W2, z((32, 32)), z((32, 32))],
        [z((32, 32)), z((32, 32)), W2, z((32, 32))],
        [z((32, 32)), z((32, 32)), z((32, 32)), W2],
    ]).astype(np.float32)
    f['Wm1'] = Wm1.astype(np.float32)
    f['bm1'] = bm1.astype(np.float32).reshape(64, 1)

    W4 = Wm2.reshape(64, N_PATH, HC, HC)
    b4 = bm2.reshape(N_PATH, HC, HC)

    def wpk(k, seg):
        return (W4[:, k] @ seg).astype(np.float32), (b4[k] @ seg).astype(np.float32)

    Wp, Bp = {}, {}
    Wp[0], Bp[0] = wpk(0, P0[0:32])
    Wp[2], Bp[2] = wpk(2, P0[32:64])
    Wp[6], Bp[6] = wpk(6, P0[64:96])
    w1, b1 = wpk(1, P2[0:32])
    w5, b5 = wpk(5, P2[64:96])
    Wp[15], Bp[15] = w1 + w5, b1 + b5
    Wp[4], Bp[4] = wpk(4, P2[32:64])
    Wp[8], Bp[8] = wpk(8, P2[96:128])

    order = [0, 15, 15, 15, 15, 15, 2, 4, 6, 8, 4, 4, 4, 4, 8, 8, 8, 8]
    Wrep = np.concatenate([Wp[k] for k in order], axis=1)          # [64,576]
    brep = np.concatenate([Bp[k] for k in order])[None, :]         # [1,576]
    f['Wm2PrepB'] = np.concatenate([Wrep, brep], axis=0).astype(np.float32)  # [65,576]

    I32 = np.eye(32, dtype=np.float32)

    def mk(K):
        return np.zeros((K, 128), dtype=np.float32)

    s2, s6 = 1 / np.sqrt(2), 1 / np.sqrt(6)
    # G1 = [p2-iso, p4-z2comp, p6-iso, p8-z2comp]; input SQB=[x2,y2,z2,s4^2]
    A2G1 = mk(128)
    for t, c in [(0, 1/np.sqrt(3)), (1, 1/np.sqrt(3)), (2, 1/np.sqrt(3))]:
        A2G1[32*t:32*t+32, 0:32] += c * I32
    for t, c in [(0, -s6), (1, -s6), (2, 2*s6)]:
        A2G1[32*t:32*t+32, 32:64] += c * I32
    A2G1[96:128, 64:96] += (1/np.sqrt(5)) * I32
    A2G1[96:128, 96:128] += C222[4, 4, 2] * I32
    C2G1 = mk(128)   # input SQ2=[s0^2..s3^2]
    for i in range(4):
        C2G1[32*i:32*i+32, 64:96] += (1/np.sqrt(5)) * I32
        C2G1[32*i:32*i+32, 96:128] += C222[i, i, 2] * I32
    # G2 = [xy', yz', xz', x2-y2'] ; input TPR=[xy, yz, xz]
    B2G2 = mk(96)
    B2G2[0:32, 0:32] += 2 * s2 * I32
    B2G2[32:64, 32:64] += 2 * s2 * I32
    B2G2[64:96, 64:96] += 2 * s2 * I32
    SQB2G2 = mk(128)  # input SQB
    SQB2G2[0:32, 96:128] += s2 * I32
    SQB2G2[32:64, 96:128] += -s2 * I32
    # G3 = [r0', r1', r3', r4']; inputs TOD=[s0s1,s1s2,s2s3,s3s4],
    # TG3B=[s0s2,s1s3,s2s4,junk], TG3C=[s0s3,s1s4], SQ2
    D2G3 = mk(128)
    D2G3[32:64, 32:64] += 2 * C222[1, 2, 1] * I32
    D2G3[0:32, 64:96] += 2 * C222[0, 1, 3] * I32
    D2G3[64:96, 64:96] += 2 * C222[2, 3, 3] * I32
    D2G3[96:128, 64:96] += 2 * C222[3, 4, 3] * I32
    E2G3 = mk(128)   # input TG3B=[s0s2,s1s3,s2s4,s3s0]
    E2G3[0:32, 0:32] += 2 * C222[0, 2, 0] * I32
    E2G3[32:64, 0:32] += 2 * C222[1, 3, 0] * I32
    E2G3[64:96, 96:128] += 2 * C222[2, 4, 4] * I32
    E2G3[96:128, 32:64] += 2 * C222[0, 3, 1] * I32
    SQ22G3 = mk(128)  # SQ2 diag -> G3 r4
    SQ22G3[32:64, 96:128] += C222[1, 1, 4] * I32
    SQ22G3[96:128, 96:128] += C222[3, 3, 4] * I32
    f.update(A2G1=A2G1, C2G1=C2G1, B2G2=B2G2, SQB2G2=SQB2G2,
             D2G3=D2G3, E2G3=E2G3, SQ22G3=SQ22G3)

    inv_u = 1.0 / np.sqrt(HC)
    c0 = inv_u / np.sqrt(3 * HC)
    c2 = inv_u / np.sqrt(4 * HC)
    ones = np.ones(32, dtype=np.float32)
    # v1 lives at partitions 64:128, so Ra sits at rows 64:128 of the packed
    # consts (matmul lhsT/rhs must share base partition)
    # VVB = [junk, s1s4*pw8, h0^2*pw0, h0s4*pw15]
    Ra = np.zeros((128, 6), dtype=np.float32)
    Ra[32:64, 2] = c2 * 2 * C222[1, 4, 1] * ones
    Ra[64:96, 0] = c0 * ones
    Ra[96:128, 5] = c2 * ones
    Rb = np.zeros((128, 6), dtype=np.float32)  # v2 = HH = [h0*s0..h0*s3]
    for r in range(4):
        Rb[32*r:32*r+32, 1 + r] = c2 * ones
    Rc = np.zeros((128, 6), dtype=np.float32)  # v3 = G1
    Rc[0:32, 0] = c0 * ones
    Rc[32:64, 3] = c2 * ones
    Rc[64:96, 0] = c0 * ones
    Rc[96:128, 3] = c2 * ones
    Rd = np.zeros((128, 6), dtype=np.float32)  # v4 = G2
    Rd[0:32, 1] = c2 * ones
    Rd[32:64, 2] = c2 * ones
    Rd[64:96, 4] = c2 * ones
    Rd[96:128, 5] = c2 * ones
    Re = Rd.copy()                              # v5 = G3 (same comp layout)
    f.update(Ra=Ra, Rb=Rb, Rc=Rc, Rd=Rd, Re=Re)
    return f


CONST_NAMES = ['Wh0R', 'Wh101', 'WhZS4', 'WhS', 'Wm1', 'Wm2PrepB',
               'A2G1', 'C2G1', 'B2G2', 'SQB2G2', 'D2G3', 'E2G3',
               'SQ22G3', 'Ra', 'Rb', 'Rc', 'Rd', 'Re']


def pack_consts(f):
    """Pack all bf16 lhsT constants into one [128, W] tensor, zero-padded to
    128 partitions; returns (packed, {name: (k, off, m)})."""
    cols = []
    offs = {}
    w = 0
    for name in CONST_NAMES:
        a = f[name]
        k, m = a.shape
        pad = np.zeros((128, m), dtype=np.float32)
        pad[:k] = a
        cols.append(pad)
        offs[name] = (k, w, m)
        w += m
    return np.concatenate(cols, axis=1).astype(BF), offs


def prep_x(x_scalar, x_spherical):
    """-> xall [128, 5, n] bf16: slot0 = x_scalar.T; slot1 = x0;
    slot2 = [x1m0,x1m1]; slot3 = [x1m2, x2r4, pad]; slot4 = [x2r0..r3]."""
    n = x_scalar.shape[0]
    x1 = x_spherical[:, 128:320].reshape(n, 64, 3)
    x2 = x_spherical[:, 320:480].reshape(n, 32, 5)
    xall = np.zeros((128, 5, n), dtype=np.float32)
    xall[:, 0] = x_scalar.T
    xall[:, 1] = x_spherical[:, 0:128].T
    xall[0:64, 2] = x1[:, :, 0].T
    xall[64:128, 2] = x1[:, :, 1].T
    xall[0:64, 3] = x1[:, :, 2].T
    xall[64:96, 3] = x2[:, :, 4].T
    xall[0:32, 4] = x2[:, :, 0].T
    xall[32:64, 4] = x2[:, :, 1].T
    xall[64:96, 4] = x2[:, :, 2].T
    xall[96:128, 4] = x2[:, :, 3].T
    # chunk-major blocks so each chunk's DMA is one contiguous blob
    blocks = [np.ascontiguousarray(xall[:, :, o:o + f]).reshape(-1)
              for o, f in zip(COFF, CHUNKS)]
    return np.concatenate(blocks).astype(BF)


# ---------------- device program ----------------
_CACHE = {}


def build_program():
    if 'nc' in _CACHE:
        return _CACHE['nc']
    import concourse.bacc as bacc
    from concourse import mybir
    from concourse.tile import TileContext

    bf = mybir.dt.bfloat16
    f32 = mybir.dt.float32
    nc = bacc.Bacc()

    offs = _CACHE['const_offs']
    CW = _CACHE['const_w']
    # columns needed by the h-stage (head-latency critical) vs the rest
    CWA = offs['Wm1'][1] + offs['Wm1'][2]

    xall = nc.declare_dram_parameter("xall", [128 * 5 * NC_PER], bf, isOutput=False)
    cpk = nc.declare_dram_parameter("cpk", [128, CW], bf, isOutput=False)
    bm1 = nc.declare_dram_parameter("bm1", [64, 1], f32, isOutput=False)
    zout = nc.declare_dram_parameter("z", [6, NC_PER], f32, isOutput=True)

    with TileContext(nc) as tc:
        with (
            tc.tile_pool(name="consts", bufs=1) as cp,
            tc.tile_pool(name="xin", bufs=4) as xp,
            tc.tile_pool(name="work", bufs=4) as wk,
            tc.tile_pool(name="ps", bufs=1, space="PSUM") as pp,
        ):
            CT = cp.tile([128, CW], bf, name="CT")
            # consts ride the gpsimd queue so the SP queue belongs to X0;
            # h-stage weights (CTa) first for minimum latency
            nc.gpsimd.dma_start(out=CT[:, 0:CWA], in_=cpk[:, 0:CWA])
            bm1_t = cp.tile([64, 1], f32, name="bm1_t")
            nc.gpsimd.dma_start(out=bm1_t[:], in_=bm1[:])
            nc.gpsimd.dma_start(out=CT[:, CWA:CW], in_=cpk[:, CWA:CW])

            def W(name):
                k, off, m = offs[name]
                return CT[0:k, off:off + m]

            z_sb = cp.tile([6, NC_PER], f32, name="z_sb")
            # PE warm-up: a stream of small matmuls (on a zeroed tile) keeps
            # the tensor engine continuously busy from ~t=300ns so the ramp
            # (3us to full clock) completes while the first X DMA is in
            # flight -- every real matmul then runs at 2.4 GHz.
            WUP = cp.tile([64, 16], bf, name="WUP")
            nc.vector.memset(WUP[:], 0.0)
            # dummy ACT ops so both activation-table loads (Copy + Silu)
            # happen at t~0 instead of lazily on the chunk-0 critical path
            nc.scalar.copy(WUP[0:1, 0:2], WUP[0:1, 2:4])
            nc.scalar.activation(WUP[0:1, 0:2], WUP[0:1, 2:4],
                                 mybir.ActivationFunctionType.Silu,
                                 bias=0.0, scale=1.0)
            # (no PE warm-up needed: pe_busy_start stays 0 until the first
            # matmul dispatch, so a first matmul after t=3us runs at full
            # clock -- chunk 0's X pieces are ordered to land just past it)
            PH4 = pp.tile([128, 4, 512], f32, name="PH4")  # h0R, hB, h2, pa+pwB5
            PW2 = pp.tile([128, 2, 512], f32, name="PW2")  # pw waves; then G1; pz
            PG2 = pp.tile([128, 2, 512], f32, name="PG2")  # G2, G3

            # aS / SHALL triple-buffers (persistent, chunk mod-3 indexed) so
            # fronts can run two chunks ahead of backs
            aS_bufs, SH_bufs = [], []
            for i in range(3):
                t = cp.tile([65, F], bf, name=f"aS{i}")
                nc.gpsimd.memset(t[64:65, :], 1.0)
                aS_bufs.append(t)
                s = cp.tile([128, 4, F], bf, name=f"SHALL{i}")
                # zero the never-copied strips so junk product rows stay finite
                (nc.gpsimd if i == 0 else nc.vector).memset(s[96:128, :, :], 0.0)
                SH_bufs.append(s)

            def load_x(c):
                # split by h-matmul consumption order (slots 1,2 / 3,4 / 0)
                # so the first h matmuls can start one DMA earlier.  For
                # chunk 0 the [3:5] piece goes first: the first matmul then
                # dispatches just after t=3us, past the PE p-state ramp, so
                # every matmul in the program runs at 2.4 GHz.
                X = xp.tile([128, 5, F], bf, tag="X")
                xoff = 128 * 5 * COFF[c]
                src = xall[xoff:xoff + 128 * 5 * F].rearrange(
                    "(p s f) -> p s f", p=128, s=5)
                pieces = [(1, 3), (3, 5), (0, 1)] if c else [(3, 5), (1, 3), (0, 1)]
                for a, b in pieces:
                    nc.sync.dma_start(out=X[:, a:b, :], in_=src[:, a:b, :])
                return X

            def stage_front(c, X):
                """h matmuls + evictions + silu + shift DMAs for chunk c."""
                nc.tensor.matmul(PH4[:, 0, 0:F], W('Wh0R'), X[:, 1, :],
                                 start=True, stop=True, skip_group_check=True)
                nc.tensor.matmul(PH4[0:64, 1, 0:F], W('Wh101'), X[:, 2, :],
                                 start=True, stop=True, skip_group_check=True)
                nc.tensor.matmul(PH4[64:128, 1, 0:F], W('WhZS4'), X[0:96, 3, :],
                                 start=True, stop=True, skip_group_check=True)
                nc.tensor.matmul(PH4[:, 2, 0:F], W('WhS'), X[:, 4, :],
                                 start=True, stop=True, skip_group_check=True)
                nc.tensor.matmul(PH4[0:64, 3, 0:F], W('Wm1'), X[:, 0, :],
                                 start=True, stop=True, skip_group_check=True)
                # HALL = [H0R | H_B | H2] bf16 (one wide eviction)
                HALL = wk.tile([128, 3, F], bf, tag="HALL")
                nc.scalar.copy(HALL[:], PH4[:, 0:3, 0:F])
                aS = aS_bufs[c % 3]
                nc.scalar.activation(aS[0:64, :], PH4[0:64, 3, 0:F],
                                     mybir.ActivationFunctionType.Silu,
                                     bias=bm1_t[:], scale=1.0)
                # shifted operand planes: slots 0/1 via DMA on the SP queue
                # (the only queue whose transfer time isn't charged to a
                # compute engine), slot2 via partition-shifted engine
                # copies (single-input ops may shift partition base).
                # SH slot0 = [y, z, x, 0]; slot1 = [s1, s2, s3, s4]
                # slot2 = [s2, s3, s4, s0]  (s0s3 folded into TG3B's mm;
                # s1s4 is a DVE psum-read product into VVB[0:32])
                SH = SH_bufs[c % 3]
                nc.sync.dma_start(out=SH[0:64, 0, :], in_=HALL[32:96, 1, :])
                nc.sync.dma_start(out=SH[64:96, 0, :], in_=HALL[0:32, 1, :])
                nc.sync.dma_start(out=SH[0:96, 1, :], in_=HALL[32:128, 2, :])
                nc.sync.dma_start(out=SH[96:128, 1, :], in_=HALL[96:128, 1, :])
                nc.gpsimd.tensor_copy(SH[0:64, 2, :], HALL[64:128, 2, :])
                nc.gpsimd.tensor_copy(SH[64:96, 2, :], HALL[96:128, 1, :])
                nc.gpsimd.tensor_copy(SH[96:128, 2, :], HALL[0:32, 2, :])
                return HALL, aS

            def stage_products(c, HALL):
                """pair products for chunk c (Pool/DVE, read HALL/SH sbuf)."""
                SH = SH_bufs[c % 3]
                H0 = HALL[:, 0, :]
                H2 = HALL[:, 2, :]
                SS = wk.tile([128, 2, F], bf, tag="SS")    # [SQB | SQ2]
                nc.vector.tensor_mul(SS[:], HALL[:, 1:3, :], HALL[:, 1:3, :])
                HHT = wk.tile([128, F], bf, tag="HHT")     # [h0*s0..h0*s3]
                nc.gpsimd.tensor_mul(HHT[:], H0, H2)
                SMT = wk.tile([128, F], bf, tag="SMT")     # [.,.,h0^2,h0*s4]
                nc.gpsimd.tensor_mul(SMT[64:96, :], HALL[64:96, 0, :],
                                     HALL[64:96, 0, :])
                nc.gpsimd.tensor_mul(SMT[96:128, :], HALL[96:128, 0, :],
                                     HALL[96:128, 1, :])
                PT = wk.tile([128, 2, F], bf, tag="PT")    # [xy,yz,xz,s4^2 | OD1]
                nc.gpsimd.tensor_mul(PT[:], HALL[:, 1:3, :], SH[:, 0:2, :])
                TG3B = wk.tile([128, F], bf, tag="TG3B")   # [s0s2,s1s3,s2s4,s3s0]
                nc.gpsimd.tensor_mul(TG3B[:], H2, SH[:, 2, :])
                # SM2[96:128] = s1*s4 (psum-side read of [s0,s1] against the
                # sbuf [hz,s4] planes; the [64:96] block is harmless junk)
                SM2 = wk.tile([128, F], bf, tag="SM2")
                nc.vector.tensor_mul(SM2[64:128, :], PH4[0:64, 2, 0:F],
                                     HALL[64:128, 1, :])
                return dict(SS=SS, HHT=HHT, SMT=SMT, PT=PT, TG3B=TG3B, SM2=SM2)

            def stage_pw1(c, aS, P):
                """pwB5 + wave1 + wave1b evict + VVB.  Emitted after
                front(c+1) but before back2(c); the pw1b evict precedes
                HALL(c+2) in ACT order (deadlock-free)."""
                WP = W('Wm2PrepB')
                nc.tensor.matmul(PH4[64:128, 3, 0:F], WP[:, 0:64], aS[:],
                                 start=True, stop=True, skip_group_check=True)
                for i in range(2):
                    nc.tensor.matmul(PW2[:, i, 0:F], WP[:, 64 + 128*i:192 + 128*i],
                                     aS[:], start=True, stop=True,
                                     skip_group_check=True)
                PWS1b = wk.tile([128, F], bf, tag="PWS1b")
                nc.scalar.copy(PWS1b[:], PW2[:, 1, 0:F])
                # VVB = [junk*pw6, s1s4*pw8, h0^2*pw0, h0s4*pw15]
                VVB = wk.tile([128, F], bf, tag="VVB")
                nc.vector.tensor_mul(VVB[64:128, :], PH4[64:128, 3, 0:F],
                                     P['SMT'][64:128, :])
                nc.vector.tensor_mul(VVB[0:64, :], P['SM2'][64:128, :],
                                     PWS1b[64:128, :])
                return dict(PWS1b=PWS1b, VVB=VVB)

            def stage_back2a(c, aS, P, Q):
                """G1 + G2/G3 + GS evict + wave2 + VV + R + output."""
                WP = W('Wm2PrepB')
                # G1 into the wave1b bank (freed by the pw1b evict)
                nc.tensor.matmul(PW2[:, 1, 0:F], W('A2G1'), P['SS'][:, 0, :],
                                 start=True, stop=False, skip_group_check=True)
                nc.tensor.matmul(PW2[:, 1, 0:F], W('C2G1'), P['SS'][:, 1, :],
                                 start=False, stop=True, skip_group_check=True)
                # VV1' = pwmix (sbuf) * G1 (psum)
                VV1p = wk.tile([128, F], bf, tag="VV1p")
                nc.vector.tensor_mul(VV1p[:], Q['PWS1b'][:], PW2[:, 1, 0:F])
                nc.tensor.matmul(PG2[:, 0, 0:F], W('B2G2'), P['PT'][0:96, 0, :],
                                 start=True, stop=False, skip_group_check=True)
                nc.tensor.matmul(PG2[:, 0, 0:F], W('SQB2G2'), P['SS'][:, 0, :],
                                 start=False, stop=True, skip_group_check=True)
                nc.tensor.matmul(PG2[:, 1, 0:F], W('D2G3'), P['PT'][:, 1, :],
                                 start=True, stop=False, skip_group_check=True)
                nc.tensor.matmul(PG2[:, 1, 0:F], W('E2G3'), P['TG3B'][:],
                                 start=False, stop=False, skip_group_check=True)
                nc.tensor.matmul(PG2[:, 1, 0:F], W('SQ22G3'), P['SS'][:, 1, :],
                                 start=False, stop=True, skip_group_check=True)
                GS = wk.tile([128, 2, F], bf, tag="GS")
                nc.scalar.copy(GS[:], PG2[:, 0:2, 0:F])
                # wave2 into the freed G banks
                for i in range(2):
                    nc.tensor.matmul(PG2[:, i, 0:F],
                                     WP[:, 320 + 128*i:448 + 128*i], aS[:],
                                     start=True, stop=True, skip_group_check=True)
                # ---- v products ----
                VV = wk.tile([128, 3, F], bf, tag="VV")
                nc.vector.tensor_mul(VV[:, 0, :], P['HHT'][:], PW2[:, 0, 0:F])
                nc.vector.tensor_mul(VV[:, 1:3, :], GS[:], PG2[:, 0:2, 0:F])
                return dict(VV=VV, VV1p=VV1p)

            def stage_back2b(c, Q, R):
                """R reduction -> pz, evict, DMA out.  Emitted one full
                iteration after back2a(c): the 5 R matmuls would otherwise
                park on the PE wait-queue (depth 4) waiting for VV23 and
                head-of-line-block every later ready matmul.
                pz lives in the pwB5 bank rows 64:70 -- its reuse loop
                (z -> pwB5 -> VVB -> R) is the shortest of any psum bank."""
                VV, VV1p = R['VV'], R['VV1p']
                pz = PH4[64:70, 3, 0:F]
                nc.tensor.matmul(pz, W('Ra'), Q['VVB'][:],
                                 start=True, stop=False, skip_group_check=True)
                nc.tensor.matmul(pz, W('Rb'), VV[:, 0, :], start=False,
                                 stop=False, skip_group_check=True)
                nc.tensor.matmul(pz, W('Rc'), VV1p[:], start=False,
                                 stop=False, skip_group_check=True)
                nc.tensor.matmul(pz, W('Rd'), VV[:, 1, :], start=False,
                                 stop=False, skip_group_check=True)
                nc.tensor.matmul(pz, W('Re'), VV[:, 2, :], start=False,
                                 stop=True, skip_group_check=True)
                sl = slice(COFF[c], COFF[c] + F)
                nc.scalar.copy(z_sb[:, sl], pz)
                nc.sync.dma_start(out=zout[:, sl], in_=z_sb[:, sl])

            # ---- software-pipelined emission, fronts 2 chunks ahead ----
            # iteration c emits: X(c+3) | front(c+2) + products(c+2) |
            # back2(c) | pw1(c+1).  Fronts depend only on the previous
            # HALL eviction; the back stages trail two chunks behind, so
            # neither starves the other on the in-order engine queues.
            Xs = [load_x(0), load_x(1)]
            ST, PR, Q, RR = {}, {}, {}, {}
            ST[0] = stage_front(0, Xs[0])
            PR[0] = stage_products(0, ST[0][0])
            Xs.append(load_x(2))
            ST[1] = stage_front(1, Xs[1])
            PR[1] = stage_products(1, ST[1][0])
            Q[0] = stage_pw1(0, ST[0][1], PR[0])
            for c in range(NCHUNK):
                if c + 3 < NCHUNK:
                    Xs.append(load_x(c + 3))
                if c + 2 < NCHUNK:
                    ST[c + 2] = stage_front(c + 2, Xs[c + 2])
                    PR[c + 2] = stage_products(c + 2, ST[c + 2][0])
                RR[c] = stage_back2a(c, ST[c][1], PR[c], Q[c])
                stage_back2b(c, Q[c], RR[c])
                if c + 1 < NCHUNK:
                    Q[c + 1] = stage_pw1(c + 1, ST[c + 1][1], PR[c + 1])

    nc.finalize()
    _CACHE['nc'] = nc
    return nc


def kernel(x_scalar, x_spherical, batch, W0e, W1o, W2e, Wm1, bm1, Wm2, bm2,
           P0, P1, P2):
    from concourse.bass_utils import run_bass_kernel_spmd
    import os

    x_scalar = np.asarray(x_scalar, dtype=np.float32)
    x_spherical = np.asarray(x_spherical, dtype=np.float32)
    batch = np.asarray(batch)
    f = fold_weights(np.asarray(W0e, np.float32), np.asarray(W1o, np.float32),
                     np.asarray(W2e, np.float32), np.asarray(Wm1, np.float32),
                     np.asarray(bm1, np.float32), np.asarray(Wm2, np.float32),
                     np.asarray(bm2, np.float32), np.asarray(P0, np.float32),
                     np.asarray(P1, np.float32), np.asarray(P2, np.float32))
    cpk, offs = pack_consts(f)
    _CACHE['const_offs'] = offs
    _CACHE['const_w'] = cpk.shape[1]

    nc = build_program()
    in_maps = []
    for c in range(N_CORES):
        sl = slice(c * NC_PER, (c + 1) * NC_PER)
        xa = prep_x(x_scalar[sl], x_spherical[sl])
        in_maps.append({"xall": xa, "cpk": cpk,
                        "bm1": f['bm1'].astype(np.float32)})

    trace = bool(int(os.environ.get("KERNEL_TRACE", "0")))
    res = run_bass_kernel_spmd(nc, in_maps, core_ids=list(range(N_CORES)),
                               trace=trace)
    _CACHE['last_results'] = res

    # host post-processing: sph -> cart -> segment_sum -> roll
    z = np.concatenate([np.asarray(r["z"], np.float64) for r in res.results],
                       axis=1)                       # [6, 20000]
    sph = np.zeros((N_NODES, 9), dtype=np.float64)
    sph[:, 0] = z[0]
    sph[:, 4:9] = z[1:6].T
    cart = np.einsum('abi,ni->nab', QC, sph)
    red = np.zeros((N_GRAPH, 3, 3), dtype=np.float64)
    np.add.at(red, batch.astype(np.int64), cart)
    out = np.roll(np.roll(red, 1, axis=1), 1, axis=2)
    return out.astype(np.float32)



# revision 50
# speedup vs baseline: 1.0657x; 1.0657x over previous
"""Trainium2 Bass kernel for nn_CartTensorOut (e3nn-style CartTensorOut layer).

Strategy (v2):
- Data-parallel over nodes: 20000 nodes -> 8 cores x 2500; 5 chunks of 500.
- Host folds post_lin vectors P into Wm2, collapsing the tensor product +
  post_lin to an 18-block bilinear form (paths 3/7 vanish, 1/5 merge).
- Device: h-planes are produced by PE matmuls laid out so every pair
  product is a partition-offset view of just three evicted planes
  (H0R|H_B|H2) -- no shift copies or shuffling DMAs at all.
- PSUM is managed as 2 alternating 4-bank "quads" (h-quad, pw-quad,
  G-quad per chunk); wide 2-bank evictions amortize fixed op costs.
- v = wp * G products run on DVE as two wide bf16 ops; the reduction to
  z[6] and the sph->cart->segment_sum finish on PE / host as before.
"""
import numpy as np
import ml_dtypes

BF = ml_dtypes.bfloat16
N_NODES = 20000
N_GRAPH = 256
N_CORES = 8
NC_PER = N_NODES // N_CORES          # 2500
F = 500                               # nodes per chunk
CHUNKS = [500, 500, 500, 500, 500]
NCHUNK = len(CHUNKS)
COFF = [sum(CHUNKS[:i]) for i in range(NCHUNK)]
HC = 32
N_PATH = 9


# ---------------- constant tables (from reference's cartesian tensor algebra) ----
def _tables():
    eps = np.zeros((3, 3, 3))
    eps[0, 1, 2] = eps[1, 2, 0] = eps[2, 0, 1] = 1.0
    eps[0, 2, 1] = eps[2, 1, 0] = eps[1, 0, 2] = -1.0
    s2, s6 = 1 / np.sqrt(2), 1 / np.sqrt(6)
    Q2 = np.zeros((5, 3, 3))
    Q2[0, 0, 1] = Q2[0, 1, 0] = s2
    Q2[1, 1, 2] = Q2[1, 2, 1] = s2
    Q2[2, 0, 0] = Q2[2, 1, 1] = -s6; Q2[2, 2, 2] = 2 * s6
    Q2[3, 0, 2] = Q2[3, 2, 0] = s2
    Q2[4, 0, 0] = s2; Q2[4, 1, 1] = -s2

    def nrm(C, l3):
        return C * np.sqrt((2 * l3 + 1) / (C ** 2).sum())

    M = np.einsum('iab,jdb->ijad', Q2, Q2)
    S = 0.5 * (M + np.transpose(M, (0, 1, 3, 2)))
    S = S - np.trace(S, axis1=2, axis2=3)[..., None, None] * np.eye(3) / 3.0
    C222 = nrm(np.einsum('kad,ijad->ijk', Q2, S), 2)
    Qc = np.zeros((3, 3, 9))
    Qc[:, :, 0] = np.eye(3) / np.sqrt(3)
    Qc[:, :, 1:4] = eps / np.sqrt(2)
    Qc[:, :, 4:9] = np.transpose(Q2, (1, 2, 0))
    return C222, Qc


C222, QC = _tables()


# ---------------- host-side weight folding ----------------
def fold_weights(W0e, W1o, W2e, Wm1, bm1, Wm2, bm2, P0, P1, P2):
    f = {}
    W0 = W0e / np.sqrt(128)
    W1 = W1o / np.sqrt(64)
    W2 = W2e / np.sqrt(32)
    z = np.zeros
    f['Wh0R'] = np.tile(W0, (1, 4)).astype(np.float32)      # [128,128] -> [h0 x4]
    f['Wh101'] = np.block([[W1, z((64, 32))], [z((64, 32)), W1]]).astype(np.float32)
    # WhZS4: K=96 rows [x1m2(64), x2r4(32)] -> outputs [z, s4]
    f['WhZS4'] = np.block([
        [W1, z((64, 32))],
        [z((32, 32)), W2],
    ]).astype(np.float32)
    # WhS: K=128 rows [x2r0..x2r3] -> outputs [s0..s3]
    f['WhS'] = np.block([
        [W2, z((32, 32)), z((32, 32)), z((32, 32))],
        [z((32, 32)), W2, z((32, 32)), z((32, 32))],
        [z((32, 32)), z((32, 32)), W2, z((32, 32))],
        [z((32, 32)), z((32, 32)), z((32, 32)), W2],
    ]).astype(np.float32)
    f['Wm1'] = Wm1.astype(np.float32)
    f['bm1'] = bm1.astype(np.float32).reshape(64, 1)

    W4 = Wm2.reshape(64, N_PATH, HC, HC)
    b4 = bm2.reshape(N_PATH, HC, HC)

    def wpk(k, seg):
        return (W4[:, k] @ seg).astype(np.float32), (b4[k] @ seg).astype(np.float32)

    Wp, Bp = {}, {}
    Wp[0], Bp[0] = wpk(0, P0[0:32])
    Wp[2], Bp[2] = wpk(2, P0[32:64])
    Wp[6], Bp[6] = wpk(6, P0[64:96])
    w1, b1 = wpk(1, P2[0:32])
    w5, b5 = wpk(5, P2[64:96])
    Wp[15], Bp[15] = w1 + w5, b1 + b5
    Wp[4], Bp[4] = wpk(4, P2[32:64])
    Wp[8], Bp[8] = wpk(8, P2[96:128])

    order = [0, 15, 15, 15, 15, 15, 2, 4, 6, 8, 4, 4, 4, 4, 8, 8, 8, 8]
    Wrep = np.concatenate([Wp[k] for k in order], axis=1)          # [64,576]
    brep = np.concatenate([Bp[k] for k in order])[None, :]         # [1,576]
    f['Wm2PrepB'] = np.concatenate([Wrep, brep], axis=0).astype(np.float32)  # [65,576]

    I32 = np.eye(32, dtype=np.float32)

    def mk(K):
        return np.zeros((K, 128), dtype=np.float32)

    s2, s6 = 1 / np.sqrt(2), 1 / np.sqrt(6)
    # G1 = [p2-iso, p4-z2comp, p6-iso, p8-z2comp]; input SQB=[x2,y2,z2,s4^2]
    A2G1 = mk(128)
    for t, c in [(0, 1/np.sqrt(3)), (1, 1/np.sqrt(3)), (2, 1/np.sqrt(3))]:
        A2G1[32*t:32*t+32, 0:32] += c * I32
    for t, c in [(0, -s6), (1, -s6), (2, 2*s6)]:
        A2G1[32*t:32*t+32, 32:64] += c * I32
    A2G1[96:128, 64:96] += (1/np.sqrt(5)) * I32
    A2G1[96:128, 96:128] += C222[4, 4, 2] * I32
    C2G1 = mk(128)   # input SQ2=[s0^2..s3^2]
    for i in range(4):
        C2G1[32*i:32*i+32, 64:96] += (1/np.sqrt(5)) * I32
        C2G1[32*i:32*i+32, 96:128] += C222[i, i, 2] * I32
    # G2 = [xy', yz', xz', x2-y2'] ; input TPR=[xy, yz, xz]
    B2G2 = mk(96)
    B2G2[0:32, 0:32] += 2 * s2 * I32
    B2G2[32:64, 32:64] += 2 * s2 * I32
    B2G2[64:96, 64:96] += 2 * s2 * I32
    SQB2G2 = mk(128)  # input SQB
    SQB2G2[0:32, 96:128] += s2 * I32
    SQB2G2[32:64, 96:128] += -s2 * I32
    # G3 = [r0', r1', r3', r4']; inputs TOD=[s0s1,s1s2,s2s3,s3s4],
    # TG3B=[s0s2,s1s3,s2s4,junk], TG3C=[s0s3,s1s4], SQ2
    D2G3 = mk(128)
    D2G3[32:64, 32:64] += 2 * C222[1, 2, 1] * I32
    D2G3[0:32, 64:96] += 2 * C222[0, 1, 3] * I32
    D2G3[64:96, 64:96] += 2 * C222[2, 3, 3] * I32
    D2G3[96:128, 64:96] += 2 * C222[3, 4, 3] * I32
    E2G3 = mk(128)
    E2G3[0:32, 0:32] += 2 * C222[0, 2, 0] * I32
    E2G3[32:64, 0:32] += 2 * C222[1, 3, 0] * I32
    E2G3[64:96, 96:128] += 2 * C222[2, 4, 4] * I32
    F2G3 = mk(64)
    F2G3[0:32, 32:64] += 2 * C222[0, 3, 1] * I32
    F2G3[32:64, 32:64] += 2 * C222[1, 4, 1] * I32
    SQ22G3 = mk(128)  # SQ2 diag -> G3 r4
    SQ22G3[32:64, 96:128] += C222[1, 1, 4] * I32
    SQ22G3[96:128, 96:128] += C222[3, 3, 4] * I32
    f.update(A2G1=A2G1, C2G1=C2G1, B2G2=B2G2, SQB2G2=SQB2G2,
             D2G3=D2G3, E2G3=E2G3, F2G3=F2G3, SQ22G3=SQ22G3)

    inv_u = 1.0 / np.sqrt(HC)
    c0 = inv_u / np.sqrt(3 * HC)
    c2 = inv_u / np.sqrt(4 * HC)
    ones = np.ones(32, dtype=np.float32)
    # v1 lives at partitions 64:128, so Ra sits at rows 64:128 of the packed
    # consts (matmul lhsT/rhs must share base partition)
    Ra = np.zeros((128, 6), dtype=np.float32)  # v1 = [h0^2, h0*s4] @ p64:128
    Ra[64:96, 0] = c0 * ones
    Ra[96:128, 5] = c2 * ones
    Rb = np.zeros((128, 6), dtype=np.float32)  # v2 = HH = [h0*s0..h0*s3]
    for r in range(4):
        Rb[32*r:32*r+32, 1 + r] = c2 * ones
    Rc = np.zeros((128, 6), dtype=np.float32)  # v3 = G1
    Rc[0:32, 0] = c0 * ones
    Rc[32:64, 3] = c2 * ones
    Rc[64:96, 0] = c0 * ones
    Rc[96:128, 3] = c2 * ones
    Rd = np.zeros((128, 6), dtype=np.float32)  # v4 = G2
    Rd[0:32, 1] = c2 * ones
    Rd[32:64, 2] = c2 * ones
    Rd[64:96, 4] = c2 * ones
    Rd[96:128, 5] = c2 * ones
    Re = Rd.copy()                              # v5 = G3 (same comp layout)
    f.update(Ra=Ra, Rb=Rb, Rc=Rc, Rd=Rd, Re=Re)
    return f


CONST_NAMES = ['Wh0R', 'Wh101', 'WhZS4', 'WhS', 'Wm1', 'Wm2PrepB',
               'A2G1', 'C2G1', 'B2G2', 'SQB2G2', 'D2G3', 'E2G3', 'F2G3',
               'SQ22G3', 'Ra', 'Rb', 'Rc', 'Rd', 'Re']


def pack_consts(f):
    """Pack all bf16 lhsT constants into one [128, W] tensor, zero-padded to
    128 partitions; returns (packed, {name: (k, off, m)})."""
    cols = []
    offs = {}
    w = 0
    for name in CONST_NAMES:
        a = f[name]
        k, m = a.shape
        pad = np.zeros((128, m), dtype=np.float32)
        pad[:k] = a
        cols.append(pad)
        offs[name] = (k, w, m)
        w += m
    return np.concatenate(cols, axis=1).astype(BF), offs


def prep_x(x_scalar, x_spherical):
    """-> xall [128, 5, n] bf16: slot0 = x_scalar.T; slot1 = x0;
    slot2 = [x1m0,x1m1]; slot3 = [x1m2, x2r4, pad]; slot4 = [x2r0..r3]."""
    n = x_scalar.shape[0]
    x1 = x_spherical[:, 128:320].reshape(n, 64, 3)
    x2 = x_spherical[:, 320:480].reshape(n, 32, 5)
    xall = np.zeros((128, 5, n), dtype=np.float32)
    xall[:, 0] = x_scalar.T
    xall[:, 1] = x_spherical[:, 0:128].T
    xall[0:64, 2] = x1[:, :, 0].T
    xall[64:128, 2] = x1[:, :, 1].T
    xall[0:64, 3] = x1[:, :, 2].T
    xall[64:96, 3] = x2[:, :, 4].T
    xall[0:32, 4] = x2[:, :, 0].T
    xall[32:64, 4] = x2[:, :, 1].T
    xall[64:96, 4] = x2[:, :, 2].T
    xall[96:128, 4] = x2[:, :, 3].T
    # chunk-major blocks so each chunk's DMA is one contiguous blob
    blocks = [np.ascontiguousarray(xall[:, :, o:o + f]).reshape(-1)
              for o, f in zip(COFF, CHUNKS)]
    return np.concatenate(blocks).astype(BF)


# ---------------- device program ----------------
_CACHE = {}


def build_program():
    if 'nc' in _CACHE:
        return _CACHE['nc']
    import concourse.bacc as bacc
    from concourse import mybir
    from concourse.tile import TileContext

    bf = mybir.dt.bfloat16
    f32 = mybir.dt.float32
    nc = bacc.Bacc()

    offs = _CACHE['const_offs']
    CW = _CACHE['const_w']
    # columns needed by the h-stage (head-latency critical) vs the rest
    CWA = offs['Wm1'][1] + offs['Wm1'][2]

    xall = nc.declare_dram_parameter("xall", [128 * 5 * NC_PER], bf, isOutput=False)
    cpk = nc.declare_dram_parameter("cpk", [128, CW], bf, isOutput=False)
    bm1 = nc.declare_dram_parameter("bm1", [64, 1], f32, isOutput=False)
    zout = nc.declare_dram_parameter("z", [6, NC_PER], f32, isOutput=True)

    with TileContext(nc) as tc:
        with (
            tc.tile_pool(name="consts", bufs=1) as cp,
            tc.tile_pool(name="xin", bufs=4) as xp,
            tc.tile_pool(name="work", bufs=4) as wk,
            tc.tile_pool(name="ps", bufs=1, space="PSUM") as pp,
        ):
            CT = cp.tile([128, CW], bf, name="CT")
            # consts ride the gpsimd queue so the SP queue belongs to X0;
            # h-stage weights (CTa) first for minimum latency
            nc.gpsimd.dma_start(out=CT[:, 0:CWA], in_=cpk[:, 0:CWA])
            bm1_t = cp.tile([64, 1], f32, name="bm1_t")
            nc.gpsimd.dma_start(out=bm1_t[:], in_=bm1[:])
            nc.gpsimd.dma_start(out=CT[:, CWA:CW], in_=cpk[:, CWA:CW])

            def W(name):
                k, off, m = offs[name]
                return CT[0:k, off:off + m]

            z_sb = cp.tile([6, NC_PER], f32, name="z_sb")
            # PE warm-up: a stream of small matmuls (on a zeroed tile) keeps
            # the tensor engine continuously busy from ~t=300ns so the ramp
            # (3us to full clock) completes while the first X DMA is in
            # flight -- every real matmul then runs at 2.4 GHz.
            WUP = cp.tile([64, 16], bf, name="WUP")
            nc.vector.memset(WUP[:], 0.0)
            # dummy ACT ops so both activation-table loads (Copy + Silu)
            # happen at t~0 instead of lazily on the chunk-0 critical path
            nc.scalar.copy(WUP[0:1, 0:2], WUP[0:1, 2:4])
            nc.scalar.activation(WUP[0:1, 0:2], WUP[0:1, 2:4],
                                 mybir.ActivationFunctionType.Silu,
                                 bias=0.0, scale=1.0)
            # (no PE warm-up needed: pe_busy_start stays 0 until the first
            # matmul dispatch, so a first matmul after t=3us runs at full
            # clock -- chunk 0's X pieces are ordered to land just past it)
            PH4 = pp.tile([128, 4, 512], f32, name="PH4")  # h0R, hB, h2, pa+pwB5
            PW2 = pp.tile([128, 2, 512], f32, name="PW2")  # pw waves; then G1; pz
            PG2 = pp.tile([128, 2, 512], f32, name="PG2")  # G2, G3

            # aS / SHALL triple-buffers (persistent, chunk mod-3 indexed) so
            # fronts can run two chunks ahead of backs
            aS_bufs, SH_bufs = [], []
            for i in range(3):
                t = cp.tile([65, F], bf, name=f"aS{i}")
                nc.gpsimd.memset(t[64:65, :], 1.0)
                aS_bufs.append(t)
                s = cp.tile([128, 4, F], bf, name=f"SHALL{i}")
                # zero the never-copied strips so junk product rows stay finite
                (nc.gpsimd if i == 0 else nc.vector).memset(s[96:128, :, :], 0.0)
                SH_bufs.append(s)

            def load_x(c):
                # split by h-matmul consumption order (slots 1,2 / 3,4 / 0)
                # so the first h matmuls can start one DMA earlier.  For
                # chunk 0 the [3:5] piece goes first: the first matmul then
                # dispatches just after t=3us, past the PE p-state ramp, so
                # every matmul in the program runs at 2.4 GHz.
                X = xp.tile([128, 5, F], bf, tag="X")
                xoff = 128 * 5 * COFF[c]
                src = xall[xoff:xoff + 128 * 5 * F].rearrange(
                    "(p s f) -> p s f", p=128, s=5)
                pieces = [(1, 3), (3, 5), (0, 1)] if c else [(3, 5), (1, 3), (0, 1)]
                for a, b in pieces:
                    nc.sync.dma_start(out=X[:, a:b, :], in_=src[:, a:b, :])
                return X

            def stage_front(c, X):
                """h matmuls + evictions + silu + shift DMAs for chunk c."""
                nc.tensor.matmul(PH4[:, 0, 0:F], W('Wh0R'), X[:, 1, :],
                                 start=True, stop=True, skip_group_check=True)
                nc.tensor.matmul(PH4[0:64, 1, 0:F], W('Wh101'), X[:, 2, :],
                                 start=True, stop=True, skip_group_check=True)
                nc.tensor.matmul(PH4[64:128, 1, 0:F], W('WhZS4'), X[0:96, 3, :],
                                 start=True, stop=True, skip_group_check=True)
                nc.tensor.matmul(PH4[:, 2, 0:F], W('WhS'), X[:, 4, :],
                                 start=True, stop=True, skip_group_check=True)
                nc.tensor.matmul(PH4[0:64, 3, 0:F], W('Wm1'), X[:, 0, :],
                                 start=True, stop=True, skip_group_check=True)
                # HALL = [H0R | H_B | H2] bf16 (one wide eviction)
                HALL = wk.tile([128, 3, F], bf, tag="HALL")
                nc.scalar.copy(HALL[:], PH4[:, 0:3, 0:F])
                aS = aS_bufs[c % 3]
                nc.scalar.activation(aS[0:64, :], PH4[0:64, 3, 0:F],
                                     mybir.ActivationFunctionType.Silu,
                                     bias=bm1_t[:], scale=1.0)
                # shifted operand planes: slots 0/1 via DMA on the SP queue
                # (the only queue whose transfer time isn't charged to a
                # compute engine), slot2 via partition-shifted engine
                # copies (single-input ops may shift partition base).
                # SH slot0 = [y, z, x, 0]; slot1 = [s1, s2, s3, s4]
                # slot2 = [s2, s3, s4, s0]  (s0s3 folded into TG3B's mm;
                # s1s4 is a DVE psum-read product into VVB[0:32])
                SH = SH_bufs[c % 3]
                nc.sync.dma_start(out=SH[0:64, 0, :], in_=HALL[32:96, 1, :])
                nc.sync.dma_start(out=SH[64:96, 0, :], in_=HALL[0:32, 1, :])
                nc.sync.dma_start(out=SH[0:96, 1, :], in_=HALL[32:128, 2, :])
                nc.sync.dma_start(out=SH[96:128, 1, :], in_=HALL[96:128, 1, :])
                nc.gpsimd.tensor_copy(SH[0:64, 2, :], HALL[64:128, 2, :])
                nc.gpsimd.tensor_copy(SH[64:96, 2, :], HALL[96:128, 1, :])
                nc.gpsimd.tensor_copy(SH[0:32, 3, :], HALL[96:128, 2, :])
                nc.gpsimd.tensor_copy(SH[32:64, 3, :], HALL[96:128, 1, :])
                return HALL, aS

            def stage_products(c, HALL):
                """pair products for chunk c (Pool/DVE, read HALL/SH sbuf)."""
                SH = SH_bufs[c % 3]
                H0 = HALL[:, 0, :]
                H2 = HALL[:, 2, :]
                SS = wk.tile([128, 2, F], bf, tag="SS")    # [SQB | SQ2]
                nc.vector.tensor_mul(SS[:], HALL[:, 1:3, :], HALL[:, 1:3, :])
                HHT = wk.tile([128, F], bf, tag="HHT")     # [h0*s0..h0*s3]
                nc.gpsimd.tensor_mul(HHT[:], H0, H2)
                SMT = wk.tile([128, F], bf, tag="SMT")     # [.,.,h0^2,h0*s4]
                nc.gpsimd.tensor_mul(SMT[64:96, :], HALL[64:96, 0, :],
                                     HALL[64:96, 0, :])
                nc.gpsimd.tensor_mul(SMT[96:128, :], HALL[96:128, 0, :],
                                     HALL[96:128, 1, :])
                PT = wk.tile([128, 2, F], bf, tag="PT")    # [xy,yz,xz,s4^2 | OD1]
                nc.gpsimd.tensor_mul(PT[:], HALL[:, 1:3, :], SH[:, 0:2, :])
                TG3B = wk.tile([128, F], bf, tag="TG3B")   # [s0s2,s1s3,s2s4,0]
                nc.gpsimd.tensor_mul(TG3B[:], H2, SH[:, 2, :])
                TG3C = wk.tile([64, F], bf, tag="TG3C")    # [s0s3,s1s4]
                nc.vector.tensor_mul(TG3C[:], HALL[0:64, 2, :], SH[0:64, 3, :])
                return dict(SS=SS, HHT=HHT, SMT=SMT, PT=PT, TG3B=TG3B,
                            TG3C=TG3C)

            def stage_pw1(c, aS, P):
                """pwB5 + wave1 + wave1b evict + VVB.  Emitted after
                front(c+1) but before back2(c); the pw1b evict precedes
                HALL(c+2) in ACT order (deadlock-free)."""
                WP = W('Wm2PrepB')
                nc.tensor.matmul(PH4[64:128, 3, 0:F], WP[:, 0:64], aS[:],
                                 start=True, stop=True, skip_group_check=True)
                for i in range(2):
                    nc.tensor.matmul(PW2[:, i, 0:F], WP[:, 64 + 128*i:192 + 128*i],
                                     aS[:], start=True, stop=True,
                                     skip_group_check=True)
                PWS1b = wk.tile([128, F], bf, tag="PWS1b")
                nc.scalar.copy(PWS1b[:], PW2[:, 1, 0:F])
                # VVB = SMT (sbuf) * pwB5 (psum)
                VVB = wk.tile([128, F], bf, tag="VVB")
                nc.vector.tensor_mul(VVB[64:128, :], PH4[64:128, 3, 0:F],
                                     P['SMT'][64:128, :])
                return dict(PWS1b=PWS1b, VVB=VVB)

            def stage_back2a(c, aS, P, Q):
                """G1 + G2/G3 + GS evict + wave2 + VV + R + output."""
                WP = W('Wm2PrepB')
                # G1 into the wave1b bank (freed by the pw1b evict)
                nc.tensor.matmul(PW2[:, 1, 0:F], W('A2G1'), P['SS'][:, 0, :],
                                 start=True, stop=False, skip_group_check=True)
                nc.tensor.matmul(PW2[:, 1, 0:F], W('C2G1'), P['SS'][:, 1, :],
                                 start=False, stop=True, skip_group_check=True)
                # VV1' = pwmix (sbuf) * G1 (psum)
                VV1p = wk.tile([128, F], bf, tag="VV1p")
                nc.vector.tensor_mul(VV1p[:], Q['PWS1b'][:], PW2[:, 1, 0:F])
                nc.tensor.matmul(PG2[:, 0, 0:F], W('B2G2'), P['PT'][0:96, 0, :],
                                 start=True, stop=False, skip_group_check=True)
                nc.tensor.matmul(PG2[:, 0, 0:F], W('SQB2G2'), P['SS'][:, 0, :],
                                 start=False, stop=True, skip_group_check=True)
                nc.tensor.matmul(PG2[:, 1, 0:F], W('D2G3'), P['PT'][:, 1, :],
                                 start=True, stop=False, skip_group_check=True)
                nc.tensor.matmul(PG2[:, 1, 0:F], W('E2G3'), P['TG3B'][:],
                                 start=False, stop=False, skip_group_check=True)
                nc.tensor.matmul(PG2[:, 1, 0:F], W('F2G3'), P['TG3C'][:],
                                 start=False, stop=False, skip_group_check=True)
                nc.tensor.matmul(PG2[:, 1, 0:F], W('SQ22G3'), P['SS'][:, 1, :],
                                 start=False, stop=True, skip_group_check=True)
                GS = wk.tile([128, 2, F], bf, tag="GS")
                nc.scalar.copy(GS[:], PG2[:, 0:2, 0:F])
                # wave2 into the freed G banks
                for i in range(2):
                    nc.tensor.matmul(PG2[:, i, 0:F],
                                     WP[:, 320 + 128*i:448 + 128*i], aS[:],
                                     start=True, stop=True, skip_group_check=True)
                # ---- v products ----
                VV = wk.tile([128, 3, F], bf, tag="VV")
                nc.vector.tensor_mul(VV[:, 0, :], P['HHT'][:], PW2[:, 0, 0:F])
                nc.vector.tensor_mul(VV[:, 1:3, :], GS[:], PG2[:, 0:2, 0:F])
                return dict(VV=VV, VV1p=VV1p)

            def stage_back2b(c, Q, R):
                """R reduction -> pz, evict, DMA out.  Emitted one full
                iteration after back2a(c): the 5 R matmuls would otherwise
                park on the PE wait-queue (depth 4) waiting for VV23 and
                head-of-line-block every later ready matmul.
                pz lives in the pwB5 bank rows 64:70 -- its reuse loop
                (z -> pwB5 -> VVB -> R) is the shortest of any psum bank."""
                VV, VV1p = R['VV'], R['VV1p']
                pz = PH4[64:70, 3, 0:F]
                kra, ora, mra = offs['Ra']
                nc.tensor.matmul(pz, CT[64:128, ora:ora + mra],
                                 Q['VVB'][64:128, :],
                                 start=True, stop=False, skip_group_check=True)
                nc.tensor.matmul(pz, W('Rb'), VV[:, 0, :], start=False,
                                 stop=False, skip_group_check=True)
                nc.tensor.matmul(pz, W('Rc'), VV1p[:], start=False,
                                 stop=False, skip_group_check=True)
                nc.tensor.matmul(pz, W('Rd'), VV[:, 1, :], start=False,
                                 stop=False, skip_group_check=True)
                nc.tensor.matmul(pz, W('Re'), VV[:, 2, :], start=False,
                                 stop=True, skip_group_check=True)
                sl = slice(COFF[c], COFF[c] + F)
                nc.scalar.copy(z_sb[:, sl], pz)
                nc.sync.dma_start(out=zout[:, sl], in_=z_sb[:, sl])

            # ---- software-pipelined emission, fronts 2 chunks ahead ----
            # iteration c emits: X(c+3) | front(c+2) + products(c+2) |
            # back2(c) | pw1(c+1).  Fronts depend only on the previous
            # HALL eviction; the back stages trail two chunks behind, so
            # neither starves the other on the in-order engine queues.
            Xs = [load_x(0), load_x(1)]
            ST, PR, Q, RR = {}, {}, {}, {}
            ST[0] = stage_front(0, Xs[0])
            PR[0] = stage_products(0, ST[0][0])
            Xs.append(load_x(2))
            ST[1] = stage_front(1, Xs[1])
            PR[1] = stage_products(1, ST[1][0])
            Q[0] = stage_pw1(0, ST[0][1], PR[0])
            for c in range(NCHUNK):
                if c + 3 < NCHUNK:
                    Xs.append(load_x(c + 3))
                if c + 2 < NCHUNK:
                    ST[c + 2] = stage_front(c + 2, Xs[c + 2])
                    PR[c + 2] = stage_products(c + 2, ST[c + 2][0])
                RR[c] = stage_back2a(c, ST[c][1], PR[c], Q[c])
                stage_back2b(c, Q[c], RR[c])
                if c + 1 < NCHUNK:
                    Q[c + 1] = stage_pw1(c + 1, ST[c + 1][1], PR[c + 1])

    nc.finalize()
    _CACHE['nc'] = nc
    return nc


def kernel(x_scalar, x_spherical, batch, W0e, W1o, W2e, Wm1, bm1, Wm2, bm2,
           P0, P1, P2):
    from concourse.bass_utils import run_bass_kernel_spmd
    import os

    x_scalar = np.asarray(x_scalar, dtype=np.float32)
    x_spherical = np.asarray(x_spherical, dtype=np.float32)
    batch = np.asarray(batch)
    f = fold_weights(np.asarray(W0e, np.float32), np.asarray(W1o, np.float32),
                     np.asarray(W2e, np.float32), np.asarray(Wm1, np.float32),
                     np.asarray(bm1, np.float32), np.asarray(Wm2, np.float32),
                     np.asarray(bm2, np.float32), np.asarray(P0, np.float32),
                     np.asarray(P1, np.float32), np.asarray(P2, np.float32))
    cpk, offs = pack_consts(f)
    _CACHE['const_offs'] = offs
    _CACHE['const_w'] = cpk.shape[1]

    nc = build_program()
    in_maps = []
    for c in range(N_CORES):
        sl = slice(c * NC_PER, (c + 1) * NC_PER)
        xa = prep_x(x_scalar[sl], x_spherical[sl])
        in_maps.append({"xall": xa, "cpk": cpk,
                        "bm1": f['bm1'].astype(np.float32)})

    trace = bool(int(os.environ.get("KERNEL_TRACE", "0")))
    res = run_bass_kernel_spmd(nc, in_maps, core_ids=list(range(N_CORES)),
                               trace=trace)
    _CACHE['last_results'] = res

    # host post-processing: sph -> cart -> segment_sum -> roll
    z = np.concatenate([np.asarray(r["z"], np.float64) for r in res.results],
                       axis=1)                       # [6, 20000]
    sph = np.zeros((N_NODES, 9), dtype=np.float64)
    sph[:, 0] = z[0]
    sph[:, 4:9] = z[1:6].T
    cart = np.einsum('abi,ni->nab', QC, sph)
    red = np.zeros((N_GRAPH, 3, 3), dtype=np.float64)
    np.add.at(red, batch.astype(np.int64), cart)
    out = np.roll(np.roll(red, 1, axis=1), 1, axis=2)
    return out.astype(np.float32)

